# revision 1
# baseline (speedup 1.0000x reference)
"""Chamfer distance loss kernel for Trainium2 (Bass/Tile), 8-core SPMD.

Reference computation:
    bin_centers = 0.5*(bin_edges[:, :-1] + bin_edges[:, 1:])      # [B, K=256]
    t = target.reshape(B, -1)                                     # [B, N=76800]
    d2[b,i,j] = (t[b,i] - c[b,j])^2
    dir1[b] = sum_j min_i d2[b,i,j]     (centers -> nearest target)
    dir2[b] = sum_i min_j d2[b,i,j]     (targets -> nearest center)
    out = mean_b(dir1[b] + dir2[b])

Sharding: targets (N) are split across the 8 cores (B=2 is too small to
data-parallel over batch).  Each core computes, for its 9600-target shard
and each batch:
  - dir2 partial:  sum over its targets of min_j d2   -> [B, 128, 1]
  - dir1 partial:  running min over its targets, per center -> [B, 128, 256]
Host combines: min over (cores, partitions) for dir1, sum for dir2.
"""

import sys

if "/opt/trn_rl_repo" not in sys.path:
    sys.path.insert(0, "/opt/trn_rl_repo")

import numpy as np

import concourse.bass as bass
import concourse.tile as tile
from concourse import bacc, mybir
from concourse.bass_utils import run_bass_kernel_spmd

B = 2
N = 76800          # 240*320 targets per batch
E = 257            # bin edges
K = 256            # bin centers
NCORES = 8
NSH = N // NCORES  # 9600 targets per core per batch
P = 128
COLS = NSH // P    # 75 target-columns per core per batch
GRP = 5            # columns per DVE group (must divide COLS)
NGRP = COLS // GRP

F32 = mybir.dt.float32
BIG = 1.0e30


def _build_kernel(nc, tc, t_in, e_in, dir1_out, dir2_out):
    from contextlib import ExitStack

    ctx = ExitStack()
    const_pool = ctx.enter_context(tc.tile_pool(name="const", bufs=2))
    work_pool = ctx.enter_context(tc.tile_pool(name="work", bufs=3))
    acc_pool = ctx.enter_context(tc.tile_pool(name="acc", bufs=2))

    for b in range(B):
        # ---- load inputs ----
        e_b = const_pool.tile([P, E], F32, tag="edges")
        nc.sync.dma_start(e_b[:], e_in[b : b + 1, :].to_broadcast((P, E)))

        t_sb = const_pool.tile([P, COLS], F32, tag="tsb")
        nc.sync.dma_start(t_sb[:], t_in[b])

        # centers broadcast to all partitions: c = 0.5*(e[:-1] + e[1:])
        c_sum = const_pool.tile([P, K], F32, tag="csum")
        nc.vector.tensor_add(c_sum[:], e_b[:, 0:K], e_b[:, 1:E])
        c_bcast = const_pool.tile([P, K], F32, tag="cb")
        nc.vector.tensor_scalar_mul(c_bcast[:], c_sum[:], 0.5)

        negt = const_pool.tile([P, COLS], F32, tag="negt")
        nc.vector.tensor_scalar_mul(negt[:], t_sb[:], -1.0)

        # ---- accumulators ----
        # per-target min over centers, one column per target-column
        d2min = acc_pool.tile([P, COLS], F32, tag="d2min")
        # running per-center min, GRP independent lanes, ping-pong pair
        m_a = acc_pool.tile([P, GRP, K], F32, tag="ma")
        m_b = acc_pool.tile([P, GRP, K], F32, tag="mb")
        nc.vector.memset(m_a[:], BIG)
        nc.vector.memset(m_b[:], BIG)

        for g in range(NGRP):
            d2g = work_pool.tile([P, GRP, K], F32, tag="d2g")
            for jj in range(GRP):
                j = g * GRP + jj
                # d2 = (c - t_j)^2 ; bias is the per-partition -t_j column
                nc.scalar.activation(
                    d2g[:, jj, :],
                    c_bcast[:],
                    mybir.ActivationFunctionType.Square,
                    bias=negt[:, j : j + 1],
                    scale=1.0,
                )
            # dir2: per-target min over the K centers
            nc.vector.tensor_reduce(
                out=d2min[:, g * GRP : (g + 1) * GRP],
                in_=d2g[:],
                op=mybir.AluOpType.min,
                axis=mybir.AxisListType.X,
            )
            # dir1: running min per center (GRP independent accumulator lanes)
            m = m_a if g % 2 == 0 else m_b
            nc.vector.tensor_tensor(m[:], m[:], d2g[:], op=mybir.AluOpType.min)

        # combine ping-pong accumulators
        nc.vector.tensor_tensor(m_a[:], m_a[:], m_b[:], op=mybir.AluOpType.min)
        # fold the GRP accumulator lanes: view [P, GRP, K] as [P, K, GRP]
        m_t = m_a[:].transpose([0, 2, 1])  # strided view, reduce innermost
        m_fin = acc_pool.tile([P, K], F32, tag="mfin")
        nc.vector.tensor_reduce(
            out=m_fin[:],
            in_=m_t,
            op=mybir.AluOpType.min,
            axis=mybir.AxisListType.X,
        )
        # dir2 partial sum over this core's targets (per partition)
        d2sum = acc_pool.tile([P, 1], F32, tag="d2sum")
        nc.vector.tensor_reduce(
            out=d2sum[:],
            in_=d2min[:],
            op=mybir.AluOpType.add,
            axis=mybir.AxisListType.X,
        )

        nc.sync.dma_start(dir1_out[b], m_fin[:])
        nc.sync.dma_start(dir2_out[b], d2sum[:])

    ctx.close()


_CACHE = {}


def _get_compiled():
    if "nc" in _CACHE:
        return _CACHE["nc"]
    nc = bacc.Bacc(
        "TRN2",
        target_bir_lowering=False,
        debug=False,
        enable_asserts=False,
        num_devices=NCORES,
    )
    t_in = nc.dram_tensor("t", [B, P, COLS], F32, kind="ExternalInput").ap()
    e_in = nc.dram_tensor("edges", [B, E], F32, kind="ExternalInput").ap()
    dir1_out = nc.dram_tensor("dir1", [B, P, K], F32, kind="ExternalOutput").ap()
    dir2_out = nc.dram_tensor("dir2", [B, P, 1], F32, kind="ExternalOutput").ap()

    with tile.TileContext(nc) as tc:
        _build_kernel(nc, tc, t_in, e_in, dir1_out, dir2_out)
    nc.compile()
    _CACHE["nc"] = nc
    return nc


def kernel(target: np.ndarray, bin_edges: np.ndarray) -> np.ndarray:
    target = np.asarray(target, dtype=np.float32)
    bin_edges = np.asarray(bin_edges, dtype=np.float32)
    assert target.shape == (2, 1, 240, 320) and bin_edges.shape == (B, E)

    t_flat = target.reshape(B, N)
    in_maps = []
    for c in range(NCORES):
        shard = t_flat[:, c * NSH : (c + 1) * NSH].reshape(B, P, COLS)
        in_maps.append({"t": np.ascontiguousarray(shard), "edges": bin_edges})

    nc = _get_compiled()
    res = run_bass_kernel_spmd(nc, in_maps, list(range(NCORES))).results

    dir1 = np.stack([r["dir1"] for r in res])  # [NCORES, B, P, K]
    dir2 = np.stack([r["dir2"] for r in res])  # [NCORES, B, P, 1]

    per_center = dir1.min(axis=(0, 2))             # [B, K]
    d1 = per_center.sum(axis=1, dtype=np.float64)  # [B]
    d2 = dir2.sum(axis=(0, 2, 3), dtype=np.float64)  # [B]
    out = np.float32((d1 + d2).mean())
    return np.asarray(out, dtype=np.float32)
